# revision 1
# baseline (speedup 1.0000x reference)
"""Bass/Trainium2 kernel for nn_BigramLM (dense transformer, 8 NeuronCores).

Sharding: cores (2b, 2b+1) both run the transformer body for batch b
(data-parallel over the 4 batches, pair-replicated); the final vocab
projection is split per pair member (V/2 = 16000 columns each).

Design notes (v3):
  - LN gamma/beta folded into weights host-side (kernel LN = xhat only)
  - attention fused in SBUF, causal-trimmed tiles, affine_select masking
  - per-head softmax denominators; 1/L folded into the Wo-psum evacuation
    scale (no layer-end normalize pass)
  - FFN: W2 psum accumulation over all 32 f-tiles with identity-matmul
    residual + b2 fold; h rewritten straight from psum
  - PSUM tags: sc(2) y(1) misc(1) ff(4) = 8 banks
  - q/k/v bf16 activations, f32r weights; vocab matmul bf16; logits bf16
"""

import sys

sys.path.insert(0, "/opt/trn_rl_repo")

import numpy as np

import concourse.bass as bass
import concourse.mybir as mybir
import concourse.tile as tile
from concourse import bacc
from concourse.bass_utils import run_bass_kernel_spmd
from concourse.masks import make_identity

F32 = mybir.dt.float32
F32R = mybir.dt.float32r
BF16 = mybir.dt.bfloat16
AF = mybir.ActivationFunctionType
ALU = mybir.AluOpType

V, D, H, KD, B, T = 32000, 1024, 16, 64, 4, 1024
F = 4 * D
LAYERS = 4
P = 128
NT = T // P            # 8 token tiles
NTC = T // 512         # 2 token chunks (matmul free dim)
ND = D // P            # 8 d tiles
NF = F // P            # 32 f tiles
NPAIR = H // 2
NQUAD = H // 4
VSH = V // 2           # 16000 vocab columns per core
VC = 500               # vocab chunk (psum free dim)
NVC = VSH // VC        # 32
EPS = 1e-5
SCALE = 1.0 / float(np.sqrt(KD))


def _dram_ap(handle, offset, pattern):
    t = getattr(handle, "tensor", handle)
    offset = offset + getattr(handle, "offset", 0)
    return bass.AP(tensor=t, offset=offset, ap=[list(p) for p in pattern])


def build_program():
    nc = bacc.Bacc("TRN2", target_bir_lowering=False, debug=False, num_devices=8)

    tn = {}
    tn["x_idx"] = nc.dram_tensor("x_idx", [T, 1], mybir.dt.int32, kind="ExternalInput")
    tn["tok_emb"] = nc.dram_tensor("tok_emb", [V, D], F32R, kind="ExternalInput")
    tn["pos_emb"] = nc.dram_tensor("pos_emb", [T, D], F32R, kind="ExternalInput")
    for nm, shp, dt in (
        ("wq_cat", [D, H * KD], F32R), ("wk_cat", [D, H * KD], F32R),
        ("wv_cat", [D, H * KD], F32R),
        ("bq_pair", [P, NPAIR], F32), ("bk_pair", [P, NPAIR], F32),
        ("bv_row", [1, H * KD], F32R),
        ("wo_aug", [KD + 1, H, KD], F32R),
        ("w1", [D, F], F32R), ("b1_col", [P, NF], F32),
        ("w2", [F, D], F32R), ("b2_row", [1, D], F32R),
        ("wout", [D, VSH], BF16), ("bout", [1, VSH], F32),
    ):
        tn[nm] = nc.dram_tensor(nm, shp, dt, kind="ExternalInput")
    tn["logits"] = nc.dram_tensor("logits", [T, VSH], BF16, kind="ExternalOutput")

    with tile.TileContext(nc) as tc:
        _body(nc, tc, tn)
    nc.compile()
    return nc


def _body(nc, tc, tn):
    const = tc.alloc_tile_pool(name="const", bufs=1)
    pers = tc.alloc_tile_pool(name="pers", bufs=1)
    big = tc.alloc_tile_pool(name="big", bufs=1)
    small = tc.alloc_tile_pool(name="small", bufs=3)
    ps = tc.alloc_tile_pool(name="ps", bufs=1, space="PSUM")
    _static_pools = [const, pers, big, small, ps]

    # psum tags: sc(3) y(1) misc(1) ff(3) = 8 banks
    PS_BUFS = {"sc": 3, "y": 2, "misc": 1, "ff": 2}

    def ps_tile(tag, shape, dt=F32):
        return ps.tile(shape, dt, tag=tag, bufs=PS_BUFS[tag], name="ps_" + tag)

    class TagRotor:
        def __init__(self, tags):
            self.tags = tags
            self.i = 0

        def tile(self, shape, dt=F32):
            t = self.tags[self.i % len(self.tags)]
            self.i += 1
            return ps_tile(t, shape, dt)

    # ---------------- constants ----------------
    ident_f = const.tile([P, P], F32, tag="ident_f")
    make_identity(nc, ident_f)
    ident = const.tile([P, P], F32R, tag="ident")
    nc.scalar.copy(out=ident[:], in_=ident_f[:])
    eps_t = const.tile([P, 1], F32, tag="eps_t")
    nc.vector.memset(eps_t, EPS)
    ones1_f = const.tile([1, P], F32, tag="ones1_f")
    nc.vector.memset(ones1_f, 1.0)
    ones1 = const.tile([1, P], F32R, tag="ones1")
    nc.scalar.copy(out=ones1[:], in_=ones1_f[:])

    bq_sb = const.tile([P, NPAIR], F32, tag="bq_sb")
    bk_sb = const.tile([P, NPAIR], F32, tag="bk_sb")
    b1_sb = const.tile([P, NF], F32, tag="b1_sb")
    nc.sync.dma_start(out=bq_sb, in_=tn["bq_pair"][:, :])
    nc.sync.dma_start(out=bk_sb, in_=tn["bk_pair"][:, :])
    nc.sync.dma_start(out=b1_sb, in_=tn["b1_col"][:, :])
    bv_row = const.tile([1, H * KD], F32R, tag="bv_row")
    nc.sync.dma_start(out=bv_row, in_=tn["bv_row"][:, :])
    b2_row = const.tile([1, D], F32R, tag="b2_row")
    nc.sync.dma_start(out=b2_row, in_=tn["b2_row"][:, :])
    wo_aug = const.tile([KD + 1, H, KD], F32R, tag="wo_aug")
    nc.sync.dma_start(out=wo_aug, in_=tn["wo_aug"][:, :, :])

    # ---------------- persistent activations ----------------
    h_sb = pers.tile([P, NT, D], F32R, tag="h_sb")
    xnT = pers.tile([P, ND, T], F32R, tag="xnT")

    # ---------------- embedding ----------------
    for it in range(NT):
        idx_t = small.tile([P, 1], mybir.dt.int32, tag="idx")
        nc.sync.dma_start(out=idx_t, in_=tn["x_idx"][it * P:(it + 1) * P, :])
        nc.gpsimd.indirect_dma_start(
            out=h_sb[:, it, :], out_offset=None, in_=tn["tok_emb"][:, :],
            in_offset=bass.IndirectOffsetOnAxis(ap=idx_t[:, :1], axis=0))
        pos_t = small.tile([P, D], F32R, tag="pos", bufs=1)
        nc.sync.dma_start(out=pos_t, in_=tn["pos_emb"][it * P:(it + 1) * P, :])
        nc.vector.tensor_add(out=h_sb[:, it, :], in0=h_sb[:, it, :], in1=pos_t[:])

    # -------- layer norm (gamma/beta pre-folded into weights): xhat -> outT --
    tp_rotor = TagRotor(["sc", "sc"])

    def layer_norm_transpose(out_t, it_lo=0, it_hi=NT):
        for it in range(it_lo, it_hi):
            stats = small.tile([P, 2, 6], F32, tag="bnst")
            mv = small.tile([P, 2], F32, tag="bnmv")
            for sg in range(2):
                nc.vector.bn_stats(out=stats[:, sg, :],
                                   in_=h_sb[:, it, sg * 512:(sg + 1) * 512])
            nc.vector.bn_aggr(out=mv, in_=stats)
            rstd = small.tile([P, 1], F32, tag="rstd")
            nc.scalar.activation(out=rstd, in_=mv[:, 1:2], func=AF.Sqrt,
                                 bias=eps_t[:, :], scale=1.0)
            nc.vector.reciprocal(out=rstd, in_=rstd)
            for id2 in range(ND // 2):
                idd = 2 * id2
                xt = small.tile([P, 2, P], F32R, tag="xt")
                nc.gpsimd.tensor_scalar(
                    out=xt, in0=h_sb[:, it, idd * P:(idd + 2) * P]
                    .rearrange("p (a b) -> p a b", a=2),
                    scalar1=mv[:, 0:1], scalar2=rstd,
                    op0=ALU.subtract, op1=ALU.mult)
                tp = tp_rotor.tile([P, 2, P], F32R)
                for j in range(2):
                    nc.tensor.transpose(out=tp[:, j, :], in_=xt[:, j, :],
                                        identity=ident[:])
                nc.scalar.copy(
                    out=out_t[:, idd:idd + 2, it * P:(it + 1) * P],
                    in_=tp[:])

    # causal trim: for score tile (st, tcc) the exact needed cols are
    # [st*128 - tcc*512, 512); the score matmul itself is padded to >= 256
    # cols to stay at the fp32r 1-cycle/row rate.
    def trim_lo(st, tcc):
        return max(0, st * P - tcc * 512)

    def resid_ln2(acc, it_lo, it_hi):
        for it in range(it_lo, it_hi):
            nc.vector.tensor_add(out=h_sb[:, it, :], in0=h_sb[:, it, :],
                                 in1=acc[:, it, :])
            layer_norm_transpose(xnT, it, it + 1)

    # ---------------- transformer layers (tied weights) ----------------
    for _layer in range(LAYERS):
        acc = big.tile([P, NT, D], F32R, tag="big", name="acc")
        layer_norm_transpose(xnT)

        qkvw = tc.alloc_tile_pool(name="qkvw", bufs=1)
        attn = tc.alloc_tile_pool(name="attn", bufs=1)
        proj_rotor = TagRotor(["misc", "y"])
        for quad in range(NQUAD):
            # ---- quad weights (4 heads = 256 cols) ----
            wq_t = qkvw.tile([P, ND, 256], F32R, tag="wq")
            wk_t = qkvw.tile([P, ND, 256], F32R, tag="wk")
            wv_t = qkvw.tile([P, ND, 256], F32R, tag="wv")
            for wt, src in ((wq_t, tn["wq_cat"]), (wk_t, tn["wk_cat"]),
                            (wv_t, tn["wv_cat"])):
                nc.sync.dma_start(out=wt, in_=_dram_ap(
                    src, quad * 256,
                    [[H * KD, P], [P * H * KD, ND], [1, 256]]))

            # ---- q/k projections -> [kd(2 heads), T] bf16 per pair ----
            q_sb = attn.tile([P, 2, T], BF16, tag="q_sb", bufs=2)
            k_sb = attn.tile([P, 2, T], BF16, tag="k_sb", bufs=2)
            for wt, bsb, dst in ((wq_t, bq_sb, q_sb), (wk_t, bk_sb, k_sb)):
                for pr in range(2):
                    hp = 2 * quad + pr
                    for tcc in range(NTC):
                        pp = proj_rotor.tile([P, 512])
                        for idd in range(ND):
                            nc.tensor.matmul(
                                out=pp[:],
                                lhsT=wt[:, idd, pr * P:(pr + 1) * P],
                                rhs=xnT[:, idd, tcc * 512:(tcc + 1) * 512],
                                start=(idd == 0), stop=(idd == ND - 1))
                        nc.vector.tensor_scalar(
                            out=dst[:, pr, tcc * 512:(tcc + 1) * 512],
                            in0=pp[:], scalar1=bsb[:, hp:hp + 1], scalar2=None,
                            op0=ALU.add)

            # ---- v projection -> [t, 4 heads, KD+1] bf16 (ones col) ----
            v_sb = attn.tile([P, NT, 4, KD + 1], BF16, tag="v_sb", bufs=1)
            nc.gpsimd.memset(v_sb[:, :, :, KD:KD + 1], 1.0)
            for it in range(NT):
                pv = proj_rotor.tile([P, 4 * KD])
                nc.tensor.matmul(
                    out=pv[:], lhsT=ones1[:, :],
                    rhs=bv_row[:, quad * 256:(quad + 1) * 256],
                    start=True, stop=False)
                for idd in range(ND):
                    nc.tensor.matmul(
                        out=pv[:], lhsT=xnT[:, idd, it * P:(it + 1) * P],
                        rhs=wv_t[:, idd, :],
                        start=False, stop=(idd == ND - 1))
                nc.vector.tensor_copy(
                    out=v_sb[:, it, :, 0:KD],
                    in_=pv[:].rearrange("p (a b) -> p a b", a=4))

            # ---- attention: 8 head-blocks globally sw-pipelined (depth 3) ----
            qL = attn.tile([4, T], F32R, tag="qL", bufs=2)
            blocks = [(hq, tcc) for hq in range(4) for tcc in range(NTC)]
            n_st_of = [min(NT, (tcc + 1) * 4) for hq, tcc in blocks]
            stream = [(b, st) for b in range(len(blocks))
                      for st in range(n_st_of[b])]
            pts_of = [dict() for _ in blocks]
            y_ps_of = [None] * len(blocks)

            def emit_front(b, st):
                hq, tcc = blocks[b]
                pr, hi = hq // 2, hq % 2
                lo = trim_lo(st, tcc)            # exact needed region
                w = 512 - lo
                ws = max(256, w)                 # padded score width
                s_ps = ps_tile("sc", [P, ws])
                nc.tensor.matmul(
                    out=s_ps[:],
                    lhsT=k_sb[hi * KD:(hi + 1) * KD, pr,
                              st * P:(st + 1) * P],
                    rhs=q_sb[hi * KD:(hi + 1) * KD, pr,
                             (tcc + 1) * 512 - ws:(tcc + 1) * 512],
                    start=True, stop=True)
                pt = attn.tile([P, 512], BF16, tag="pt", bufs=4)
                pad = ws - w                     # fully-masked pad cols
                if pad:
                    nc.gpsimd.memset(pt[:, 0:pad], 0.0)
                nc.scalar.activation(out=pt[:, pad:ws],
                                     in_=s_ps[:, pad:ws],
                                     func=AF.Exp, scale=SCALE)
                if st >= 4 * tcc:
                    # diagonal block: first 128 cols of the region
                    nc.gpsimd.affine_select(
                        out=pt[:, pad:pad + min(P, w)],
                        in_=pt[:, pad:pad + min(P, w)],
                        compare_op=ALU.is_ge, fill=0.0,
                        base=0, pattern=[[1, min(P, w)]],
                        channel_multiplier=-1)
                pts_of[b][st] = (pt, ws)

            def emit_back(b, st):
                hq, tcc = blocks[b]
                h_ = 4 * quad + hq
                n_st = n_st_of[b]
                if st == 0:
                    y_ps_of[b] = ps_tile("y", [KD + 1, 512])
                pt, ws = pts_of[b].pop(st)
                nc.tensor.matmul(
                    out=y_ps_of[b][:, 512 - ws:512],
                    lhsT=v_sb[:, st, hq, :], rhs=pt[:, 0:ws],
                    start=(st == 0), stop=(st == n_st - 1))
                if st != n_st - 1:
                    return
                y_sb = attn.tile([KD + 1, 512], F32R, tag="y_sb", bufs=1)
                nc.vector.tensor_copy(out=y_sb[:], in_=y_ps_of[b][:])
                # stash this head's softmax denominators (row KD of y)
                nc.sync.dma_start(
                    out=qL[hq:hq + 1, tcc * 512:(tcc + 1) * 512],
                    in_=y_sb[KD:KD + 1, :])
                o_ps = ps_tile("misc", [P, 4, KD])
                for it4 in range(4):
                    nc.tensor.matmul(
                        out=o_ps[:, it4, :],
                        lhsT=y_sb[:, it4 * P:(it4 + 1) * P],
                        rhs=wo_aug[:, h_, :], start=True, stop=True)
                nc.vector.tensor_copy(
                    out=acc[:, tcc * 4:(tcc + 1) * 4,
                            h_ * KD:(h_ + 1) * KD],
                    in_=o_ps[:])

            DEPTH = 3
            for i, (b, st) in enumerate(stream):
                emit_front(b, st)
                if i >= DEPTH:
                    emit_back(*stream[i - DEPTH])
            for i in range(len(stream) - DEPTH, len(stream)):
                emit_back(*stream[i])
            # normalize the quad's columns of acc by 1/L (deferred)
            for it in range(NT):
                ltq = ps_tile("misc", [P, 4], F32R)
                nc.tensor.transpose(out=ltq[:],
                                    in_=qL[:, it * P:(it + 1) * P],
                                    identity=ident[0:4, 0:4])
                lcq = attn.tile([P, 4], F32, tag="lcq", bufs=3)
                with nc.allow_low_precision(reason="f32-width reciprocal"):
                    nc.vector.reciprocal(out=lcq[:], in_=ltq[:])
                lbc = bass.AP(tensor=lcq.tensor, offset=lcq.offset,
                              ap=[list(lcq.ap[0]), list(lcq.ap[-1]), [0, KD]])
                qcols = acc[:, it, quad * 256:(quad + 1) * 256]
                nc.vector.tensor_tensor(
                    out=qcols.rearrange("p (a b) -> p a b", a=4),
                    in0=qcols.rearrange("p (a b) -> p a b", a=4),
                    in1=lbc, op=ALU.mult)
            if quad == NQUAD - 1:
                resid_ln2(acc, 0, 4)

        resid_ln2(acc, 4, NT)
        attn.release()
        qkvw.release()

        # ---- FFN: aT per tcc (32 f-tiles), single-sweep psum W2 ----
        ffnw = tc.alloc_tile_pool(name="ffnw", bufs=1)
        w1_rotor = TagRotor(["sc", "sc", "sc"])
        for tcc in range(NTC):
            aT = big.tile([P, NF, 512], F32R, tag="big", name="aT")
            for fg in range(NF // 2):       # 16 groups of 2 f-tiles
                w1t = ffnw.tile([P, ND, 256], F32R, tag="w1t", bufs=2)
                nc.sync.dma_start(out=w1t, in_=_dram_ap(
                    tn["w1"], fg * 256, [[F, P], [P * F, ND], [1, 256]]))
                for f2 in range(2):
                    ft = fg * 2 + f2
                    a_ps = w1_rotor.tile([P, 512])
                    for idd in range(ND):
                        nc.tensor.matmul(
                            out=a_ps[:], lhsT=w1t[:, idd, f2 * P:(f2 + 1) * P],
                            rhs=xnT[:, idd, tcc * 512:(tcc + 1) * 512],
                            start=(idd == 0), stop=(idd == ND - 1))
                    nc.scalar.activation(
                        out=aT[:, ft, :], in_=a_ps[:],
                        func=AF.Relu, bias=b1_sb[:, ft:ft + 1], scale=1.0)
            for dc in range(2):
                ff_ps = []
                for it4 in range(4):
                    it = tcc * 4 + it4
                    fp = ps_tile(["ff", "ff", "y", "misc"][it4], [P, 512])
                    ff_ps.append(fp)
                    # residual h + b2, folded into the psum accumulation
                    nc.tensor.matmul(
                        out=fp[:], lhsT=ident[:],
                        rhs=h_sb[:, it, dc * 512:(dc + 1) * 512],
                        start=True, stop=False)
                    nc.tensor.matmul(
                        out=fp[:], lhsT=ones1[:, :],
                        rhs=b2_row[:, dc * 512:(dc + 1) * 512],
                        start=False, stop=False)
                for fgrp in range(4):        # 4 groups of 8 f-tiles
                    w2t = ffnw.tile([P, 8, 512], F32R, tag="w2t", bufs=2)
                    nc.sync.dma_start(out=w2t, in_=_dram_ap(
                        tn["w2"], (fgrp * 8 * P) * D + dc * 512,
                        [[D, P], [P * D, 8], [1, 512]]))
                    for fi8 in range(8):
                        ft = fgrp * 8 + fi8
                        last = (fgrp == 3 and fi8 == 7)
                        for it4 in range(4):
                            nc.tensor.matmul(
                                out=ff_ps[it4][:],
                                lhsT=aT[:, ft, it4 * P:(it4 + 1) * P],
                                rhs=w2t[:, fi8, :],
                                start=False, stop=last)
                for it4 in range(4):
                    it = tcc * 4 + it4
                    nc.scalar.copy(
                        out=h_sb[:, it, dc * 512:(dc + 1) * 512],
                        in_=ff_ps[it4][:])
        ffnw.release()

    # ---------------- final LN + vocab projection ----------------
    xnT_bf = big.tile([P, ND, T], BF16, tag="big", name="xnT_bf")
    layer_norm_transpose(xnT_bf)
    voc = tc.alloc_tile_pool(name="voc", bufs=1)
    lg_rotor = TagRotor(["sc", "y", "ff", "misc", "sc", "ff", "y", "ff"])
    for vc in range(NVC):
        wtl = voc.tile([P, ND, VC], BF16, tag="wout", bufs=3)
        nc.sync.dma_start(out=wtl, in_=_dram_ap(
            tn["wout"], vc * VC, [[VSH, P], [P * VSH, ND], [1, VC]]))
        bout_bc = voc.tile([P, VC], F32, tag="bout", bufs=2)
        nc.sync.dma_start(out=bout_bc,
                          in_=_dram_ap(tn["bout"], vc * VC, [[0, P], [1, VC]]))
        lg_sb = voc.tile([P, NT, VC], BF16, tag="lg", bufs=2)
        for it in range(NT):
            lg_ps = lg_rotor.tile([P, VC])
            for idd in range(ND):
                nc.tensor.matmul(
                    out=lg_ps[:], lhsT=xnT_bf[:, idd, it * P:(it + 1) * P],
                    rhs=wtl[:, idd, :],
                    start=(idd == 0), stop=(idd == ND - 1))
            nc.vector.tensor_add(out=lg_sb[:, it, :], in0=lg_ps[:],
                                 in1=bout_bc[:])
        nc.sync.dma_start(
            out=_dram_ap(tn["logits"], vc * VC,
                         [[VSH, P], [P * VSH, NT], [1, VC]]),
            in_=lg_sb[:])
    voc.release()
    for _p in reversed(_static_pools):
        _p.release()


_PROGRAM = None


def _get_program():
    global _PROGRAM
    if _PROGRAM is None:
        _PROGRAM = build_program()
    return _PROGRAM


def make_in_maps(inputs):
    f = lambda k: np.ascontiguousarray(np.asarray(inputs[k], dtype=np.float32))
    x = np.asarray(inputs["x"]).astype(np.int32)          # [B, T]
    import ml_dtypes
    bf = lambda a: np.ascontiguousarray(a.astype(ml_dtypes.bfloat16))

    # fold LN gamma/beta into the consuming weights:
    #   (xhat*g + b) @ W == xhat @ (g[:,None]*W) + b @ W
    g1, b1v = f("ln1_g"), f("ln1_b")
    g2, b2v = f("ln2_g"), f("ln2_b")
    gf, bfv = f("lnf_g"), f("lnf_b")
    wq = f("Wq") * g1[None, :, None]
    wk = f("Wk") * g1[None, :, None]
    wv = f("Wv") * g1[None, :, None]
    bq = f("bq") + np.einsum("d,hdk->hk", b1v, f("Wq"))
    bk = f("bk") + np.einsum("d,hdk->hk", b1v, f("Wk"))
    bv = f("bv") + np.einsum("d,hdk->hk", b1v, f("Wv"))
    w1 = f("W1") * g2[:, None]
    b1 = f("b1") + b2v @ f("W1")
    wout = f("Wout") * gf[:, None]
    bout = f("bout") + bfv @ f("Wout")

    cat = lambda w: np.ascontiguousarray(w.transpose(1, 0, 2).reshape(D, H * KD))
    bpair = lambda b_: np.ascontiguousarray(b_.reshape(NPAIR, P).T.copy())
    wo_aug = np.concatenate([f("Wo"), f("bo").reshape(H, 1, KD)], axis=1)
    wo_aug = np.ascontiguousarray(wo_aug.transpose(1, 0, 2))   # [KD+1, H, KD]
    b1_col = np.ascontiguousarray(b1.reshape(NF, P).T.copy())  # [128, NF]

    shared = {
        "tok_emb": f("tok_emb"), "pos_emb": f("pos_emb"),
        "wq_cat": cat(wq), "wk_cat": cat(wk), "wv_cat": cat(wv),
        "bq_pair": bpair(bq), "bk_pair": bpair(bk),
        "bv_row": np.ascontiguousarray(bv.reshape(1, H * KD)),
        "wo_aug": wo_aug,
        "w1": np.ascontiguousarray(w1), "b1_col": b1_col,
        "w2": f("W2"), "b2_row": f("b2").reshape(1, D),
    }
    in_maps = []
    for c in range(8):
        b, vh = c // 2, c % 2
        m = dict(shared)
        m["x_idx"] = np.ascontiguousarray(x[b].reshape(T, 1))
        m["wout"] = bf(wout[:, vh * VSH:(vh + 1) * VSH])
        m["bout"] = np.ascontiguousarray(
            bout.reshape(1, V)[:, vh * VSH:(vh + 1) * VSH])
        in_maps.append(m)
    return in_maps


def kernel(**inputs):
    in_maps = make_in_maps(inputs)
    nc = _get_program()
    res = run_bass_kernel_spmd(nc, in_maps, core_ids=list(range(8)))
    out = np.empty((B, T, V), dtype=np.float32)
    for c in range(8):
        b, vh = c // 2, c % 2
        out[b, :, vh * VSH:(vh + 1) * VSH] = \
            np.asarray(res.results[c]["logits"]).astype(np.float32)
    return out



# revision 5
# speedup vs baseline: 1.9590x; 1.9590x over previous
"""Bass/Trainium2 kernel for nn_BigramLM (dense transformer, 8 NeuronCores).

Sharding: cores (2b, 2b+1) both run the transformer body for batch b
(data-parallel over the 4 batches, pair-replicated); the final vocab
projection is split per pair member (V/2 = 16000 columns each).

Design notes (v3):
  - LN gamma/beta folded into weights host-side (kernel LN = xhat only)
  - attention fused in SBUF, causal-trimmed tiles, affine_select masking
  - per-head softmax denominators; 1/L folded into the Wo-psum evacuation
    scale (no layer-end normalize pass)
  - FFN: W2 psum accumulation over all 32 f-tiles with identity-matmul
    residual + b2 fold; h rewritten straight from psum
  - PSUM tags: sc(2) y(1) misc(1) ff(4) = 8 banks
  - q/k/v bf16 activations, f32r weights; vocab matmul bf16; logits bf16
"""

import sys

sys.path.insert(0, "/opt/trn_rl_repo")

import numpy as np

import concourse.bass as bass
import concourse.mybir as mybir
import concourse.tile as tile
from concourse import bacc
from concourse.bass_utils import run_bass_kernel_spmd
from concourse.masks import make_identity

F32 = mybir.dt.float32
F32R = mybir.dt.float32r
BF16 = mybir.dt.bfloat16
AF = mybir.ActivationFunctionType
ALU = mybir.AluOpType

V, D, H, KD, B, T = 32000, 1024, 16, 64, 4, 1024
F = 4 * D
LAYERS = 4
P = 128
NT = T // P            # 8 token tiles
NTC = T // 512         # 2 token chunks (matmul free dim)
ND = D // P            # 8 d tiles
NF = F // P            # 32 f tiles
NPAIR = H // 2
NQUAD = H // 4
VSH = V // 2           # 16000 vocab columns per core
VC = 500               # vocab chunk (psum free dim)
NVC = VSH // VC        # 32
EPS = 1e-5
SCALE = 1.0 / float(np.sqrt(KD))


def _dram_ap(handle, offset, pattern):
    t = getattr(handle, "tensor", handle)
    offset = offset + getattr(handle, "offset", 0)
    return bass.AP(tensor=t, offset=offset, ap=[list(p) for p in pattern])


def build_program():
    nc = bacc.Bacc("TRN2", target_bir_lowering=False, debug=False, num_devices=8)

    tn = {}
    tn["x_idx"] = nc.dram_tensor("x_idx", [T, 1], mybir.dt.int32, kind="ExternalInput")
    tn["tok_emb"] = nc.dram_tensor("tok_emb", [V, D], F32R, kind="ExternalInput")
    tn["pos_emb"] = nc.dram_tensor("pos_emb", [T, D], F32R, kind="ExternalInput")
    for nm, shp, dt in (
        ("wq_cat", [D, H * KD], BF16), ("wk_cat", [D, H * KD], BF16),
        ("wv_cat", [D, H * KD], BF16),
        ("bq_pair", [P, NPAIR], F32), ("bk_pair", [P, NPAIR], F32),
        ("bv_row", [1, H * KD], F32R),
        ("wo_aug", [KD + 1, H, KD], F32R),
        ("w1", [D, F], BF16), ("b1_col", [P, NF], F32),
        ("w2", [F, D], BF16), ("b2_row", [1, D], F32R),
        ("wout", [D, VSH], BF16), ("bout", [1, VSH], F32),
    ):
        tn[nm] = nc.dram_tensor(nm, shp, dt, kind="ExternalInput")
    tn["logits"] = nc.dram_tensor("logits", [T, VSH], BF16, kind="ExternalOutput")

    with tile.TileContext(nc) as tc:
        _body(nc, tc, tn)
    nc.compile()
    return nc


def _body(nc, tc, tn):
    const = tc.alloc_tile_pool(name="const", bufs=1)
    pers = tc.alloc_tile_pool(name="pers", bufs=1)
    big = tc.alloc_tile_pool(name="big", bufs=1)
    small = tc.alloc_tile_pool(name="small", bufs=3)
    ps = tc.alloc_tile_pool(name="ps", bufs=1, space="PSUM")
    _static_pools = [const, pers, big, small, ps]

    # psum tags: sc(3) y(1) misc(1) ff(3) = 8 banks
    PS_BUFS = {"sc": 3, "y": 2, "misc": 1, "ff": 2}

    def ps_tile(tag, shape, dt=F32):
        return ps.tile(shape, dt, tag=tag, bufs=PS_BUFS[tag], name="ps_" + tag)

    class TagRotor:
        def __init__(self, tags):
            self.tags = tags
            self.i = 0

        def tile(self, shape, dt=F32):
            t = self.tags[self.i % len(self.tags)]
            self.i += 1
            return ps_tile(t, shape, dt)

    # ---------------- constants ----------------
    ident_f = const.tile([P, P], F32, tag="ident_f")
    make_identity(nc, ident_f)
    ident = const.tile([P, P], F32R, tag="ident")
    nc.scalar.copy(out=ident[:], in_=ident_f[:])
    eps_t = const.tile([P, 1], F32, tag="eps_t")
    nc.vector.memset(eps_t, EPS)
    ones1_f = const.tile([1, P], F32, tag="ones1_f")
    nc.vector.memset(ones1_f, 1.0)
    ones1 = const.tile([1, P], F32R, tag="ones1")
    nc.scalar.copy(out=ones1[:], in_=ones1_f[:])

    bq_sb = const.tile([P, NPAIR], F32, tag="bq_sb")
    bk_sb = const.tile([P, NPAIR], F32, tag="bk_sb")
    b1_sb = const.tile([P, NF], F32, tag="b1_sb")
    nc.sync.dma_start(out=bq_sb, in_=tn["bq_pair"][:, :])
    nc.sync.dma_start(out=bk_sb, in_=tn["bk_pair"][:, :])
    nc.sync.dma_start(out=b1_sb, in_=tn["b1_col"][:, :])
    bv_row = const.tile([1, H * KD], F32R, tag="bv_row")
    nc.sync.dma_start(out=bv_row, in_=tn["bv_row"][:, :])
    b2_row = const.tile([1, D], F32R, tag="b2_row")
    nc.sync.dma_start(out=b2_row, in_=tn["b2_row"][:, :])
    wo_aug = const.tile([KD + 1, H, KD], F32R, tag="wo_aug")
    nc.sync.dma_start(out=wo_aug, in_=tn["wo_aug"][:, :, :])

    # ---------------- persistent activations ----------------
    h_sb = pers.tile([P, NT, D], F32R, tag="h_sb")
    xnT = pers.tile([P, ND, T], BF16, tag="xnT")

    # ---------------- embedding ----------------
    for it in range(NT):
        idx_t = small.tile([P, 1], mybir.dt.int32, tag="idx")
        nc.sync.dma_start(out=idx_t, in_=tn["x_idx"][it * P:(it + 1) * P, :])
        nc.gpsimd.indirect_dma_start(
            out=h_sb[:, it, :], out_offset=None, in_=tn["tok_emb"][:, :],
            in_offset=bass.IndirectOffsetOnAxis(ap=idx_t[:, :1], axis=0))
        pos_t = small.tile([P, D], F32R, tag="pos", bufs=1)
        nc.sync.dma_start(out=pos_t, in_=tn["pos_emb"][it * P:(it + 1) * P, :])
        nc.vector.tensor_add(out=h_sb[:, it, :], in0=h_sb[:, it, :], in1=pos_t[:])

    # -------- layer norm (gamma/beta pre-folded into weights): xhat -> outT --
    tp_rotor = TagRotor(["sc", "sc"])

    def layer_norm_transpose(out_t, it_lo=0, it_hi=NT):
        for it in range(it_lo, it_hi):
            stats = small.tile([P, 2, 6], F32, tag="bnst")
            mv = small.tile([P, 2], F32, tag="bnmv")
            for sg in range(2):
                nc.vector.bn_stats(out=stats[:, sg, :],
                                   in_=h_sb[:, it, sg * 512:(sg + 1) * 512])
            nc.vector.bn_aggr(out=mv, in_=stats)
            rstd = small.tile([P, 1], F32, tag="rstd")
            nc.scalar.activation(out=rstd, in_=mv[:, 1:2], func=AF.Sqrt,
                                 bias=eps_t[:, :], scale=1.0)
            nc.vector.reciprocal(out=rstd, in_=rstd)
            xh = small.tile([P, D], F32R, tag="xh", bufs=2)
            nc.vector.tensor_scalar(
                out=xh, in0=h_sb[:, it, :],
                scalar1=mv[:, 0:1], scalar2=rstd,
                op0=ALU.subtract, op1=ALU.mult)
            for id2 in range(ND // 2):
                idd = 2 * id2
                tp = tp_rotor.tile([P, 2, P], F32R)
                for j in range(2):
                    nc.tensor.transpose(
                        out=tp[:, j, :],
                        in_=xh[:, (idd + j) * P:(idd + j + 1) * P],
                        identity=ident[:])
                nc.scalar.copy(
                    out=out_t[:, idd:idd + 2, it * P:(it + 1) * P],
                    in_=tp[:])

    # causal trim: for score tile (st, tcc) the exact needed cols are
    # [st*128 - tcc*512, 512); the score matmul itself is padded to >= 256
    # cols to stay at the fp32r 1-cycle/row rate.
    def trim_lo(st, tcc):
        return max(0, st * P - tcc * 512)

    def resid_ln2(acc, it_lo, it_hi):
        for it in range(it_lo, it_hi):
            nc.vector.tensor_add(out=h_sb[:, it, :], in0=h_sb[:, it, :],
                                 in1=acc[:, it, :])
            layer_norm_transpose(xnT, it, it + 1)

    # ---------------- transformer layers (tied weights) ----------------
    for _layer in range(LAYERS):
        acc = big.tile([P, NT, D], F32R, tag="big", name="acc")
        layer_norm_transpose(xnT)

        qkvw = tc.alloc_tile_pool(name="qkvw", bufs=1)
        attn = tc.alloc_tile_pool(name="attn", bufs=1)
        proj_rotor = TagRotor(["misc", "y"])
        for quad in range(NQUAD):
            # ---- quad weights (4 heads = 256 cols) ----
            wq_t = qkvw.tile([P, ND, 256], BF16, tag="wq")
            wk_t = qkvw.tile([P, ND, 256], BF16, tag="wk")
            wv_t = qkvw.tile([P, ND, 256], BF16, tag="wv")
            for wt, src in ((wq_t, tn["wq_cat"]), (wk_t, tn["wk_cat"]),
                            (wv_t, tn["wv_cat"])):
                nc.sync.dma_start(out=wt, in_=_dram_ap(
                    src, quad * 256,
                    [[H * KD, P], [P * H * KD, ND], [1, 256]]))

            # ---- q/k projections -> [kd(2 heads), T] bf16 per pair ----
            q_sb = attn.tile([P, 2, T], BF16, tag="q_sb", bufs=2)
            k_sb = attn.tile([P, 2, T], BF16, tag="k_sb", bufs=2)
            for wt, bsb, dst in ((wq_t, bq_sb, q_sb), (wk_t, bk_sb, k_sb)):
                for pr in range(2):
                    hp = 2 * quad + pr
                    for tcc in range(NTC):
                        pp = proj_rotor.tile([P, 512])
                        for idd in range(ND):
                            nc.tensor.matmul(
                                out=pp[:],
                                lhsT=wt[:, idd, pr * P:(pr + 1) * P],
                                rhs=xnT[:, idd, tcc * 512:(tcc + 1) * 512],
                                start=(idd == 0), stop=(idd == ND - 1))
                        nc.vector.tensor_scalar(
                            out=dst[:, pr, tcc * 512:(tcc + 1) * 512],
                            in0=pp[:], scalar1=bsb[:, hp:hp + 1], scalar2=None,
                            op0=ALU.add)

            # ---- v projection -> [t, 4 heads, KD+1] bf16 (ones col) ----
            v_sb = attn.tile([P, NT, 4, KD + 1], BF16, tag="v_sb", bufs=1)
            nc.gpsimd.memset(v_sb[:, :, :, KD:KD + 1], 1.0)
            for it in range(NT):
                pv = proj_rotor.tile([P, 4 * KD])
                nc.tensor.matmul(
                    out=pv[:], lhsT=ones1[:, :],
                    rhs=bv_row[:, quad * 256:(quad + 1) * 256],
                    start=True, stop=False)
                for idd in range(ND):
                    nc.tensor.matmul(
                        out=pv[:], lhsT=xnT[:, idd, it * P:(it + 1) * P],
                        rhs=wv_t[:, idd, :],
                        start=False, stop=(idd == ND - 1))
                nc.vector.tensor_copy(
                    out=v_sb[:, it, :, 0:KD],
                    in_=pv[:].rearrange("p (a b) -> p a b", a=4))

            # ---- attention: 8 head-blocks globally sw-pipelined (depth 3) ----
            qL = attn.tile([4, T], F32R, tag="qL", bufs=2)
            blocks = [(hq, tcc) for hq in range(4) for tcc in range(NTC)]
            n_st_of = [min(NT, (tcc + 1) * 4) for hq, tcc in blocks]
            stream = [(b, st) for b in range(len(blocks))
                      for st in range(n_st_of[b])]
            pts_of = [dict() for _ in blocks]
            y_ps_of = [None] * len(blocks)

            def emit_front(b, st):
                hq, tcc = blocks[b]
                pr, hi = hq // 2, hq % 2
                lo = trim_lo(st, tcc)            # exact needed region
                w = 512 - lo
                ws = max(256, w)                 # padded score width
                s_ps = ps_tile("sc", [P, ws])
                nc.tensor.matmul(
                    out=s_ps[:],
                    lhsT=k_sb[hi * KD:(hi + 1) * KD, pr,
                              st * P:(st + 1) * P],
                    rhs=q_sb[hi * KD:(hi + 1) * KD, pr,
                             (tcc + 1) * 512 - ws:(tcc + 1) * 512],
                    start=True, stop=True)
                pt = attn.tile([P, 512], BF16, tag="pt", bufs=4)
                pad = ws - w                     # fully-masked pad cols
                if pad:
                    nc.gpsimd.memset(pt[:, 0:pad], 0.0)
                nc.scalar.activation(out=pt[:, pad:ws],
                                     in_=s_ps[:, pad:ws],
                                     func=AF.Exp, scale=SCALE)
                if st >= 4 * tcc:
                    # diagonal block: first 128 cols of the region
                    nc.gpsimd.affine_select(
                        out=pt[:, pad:pad + min(P, w)],
                        in_=pt[:, pad:pad + min(P, w)],
                        compare_op=ALU.is_ge, fill=0.0,
                        base=0, pattern=[[1, min(P, w)]],
                        channel_multiplier=-1)
                pts_of[b][st] = (pt, ws)

            def emit_back(b, st):
                hq, tcc = blocks[b]
                h_ = 4 * quad + hq
                n_st = n_st_of[b]
                if st == 0:
                    y_ps_of[b] = ps_tile("y", [KD + 1, 512])
                pt, ws = pts_of[b].pop(st)
                nc.tensor.matmul(
                    out=y_ps_of[b][:, 512 - ws:512],
                    lhsT=v_sb[:, st, hq, :], rhs=pt[:, 0:ws],
                    start=(st == 0), stop=(st == n_st - 1))
                if st != n_st - 1:
                    return
                y_sb = attn.tile([KD + 1, 512], F32R, tag="y_sb", bufs=1)
                nc.vector.tensor_copy(out=y_sb[:], in_=y_ps_of[b][:])
                # stash this head's softmax denominators (row KD of y)
                nc.sync.dma_start(
                    out=qL[hq:hq + 1, tcc * 512:(tcc + 1) * 512],
                    in_=y_sb[KD:KD + 1, :])
                o_ps = ps_tile("misc", [P, 4, KD])
                for it4 in range(4):
                    nc.tensor.matmul(
                        out=o_ps[:, it4, :],
                        lhsT=y_sb[:, it4 * P:(it4 + 1) * P],
                        rhs=wo_aug[:, h_, :], start=True, stop=True)
                nc.vector.tensor_copy(
                    out=acc[:, tcc * 4:(tcc + 1) * 4,
                            h_ * KD:(h_ + 1) * KD],
                    in_=o_ps[:])

            DEPTH = 3
            for i, (b, st) in enumerate(stream):
                emit_front(b, st)
                if i >= DEPTH:
                    emit_back(*stream[i - DEPTH])
            for i in range(len(stream) - DEPTH, len(stream)):
                emit_back(*stream[i])
            # normalize the quad's columns of acc by 1/L (deferred)
            for it in range(NT):
                ltq = ps_tile("misc", [P, 4], F32R)
                nc.tensor.transpose(out=ltq[:],
                                    in_=qL[:, it * P:(it + 1) * P],
                                    identity=ident[0:4, 0:4])
                lcq = attn.tile([P, 4], F32, tag="lcq", bufs=3)
                with nc.allow_low_precision(reason="f32-width reciprocal"):
                    nc.vector.reciprocal(out=lcq[:], in_=ltq[:])
                lbc = bass.AP(tensor=lcq.tensor, offset=lcq.offset,
                              ap=[list(lcq.ap[0]), list(lcq.ap[-1]), [0, KD]])
                qcols = acc[:, it, quad * 256:(quad + 1) * 256]
                nc.vector.tensor_tensor(
                    out=qcols.rearrange("p (a b) -> p a b", a=4),
                    in0=qcols.rearrange("p (a b) -> p a b", a=4),
                    in1=lbc, op=ALU.mult)
            if quad == NQUAD - 1:
                resid_ln2(acc, 0, 4)

        resid_ln2(acc, 4, NT)
        attn.release()
        qkvw.release()

        # ---- FFN: aT per tcc (32 f-tiles), single-sweep psum W2 ----
        ffnw = tc.alloc_tile_pool(name="ffnw", bufs=1)
        w1_rotor = TagRotor(["sc", "sc", "sc"])
        for tcc in range(NTC):
            aT = big.tile([P, NF, 512], BF16, tag="big", name="aT")
            for fg in range(NF // 2):       # 16 groups of 2 f-tiles
                w1t = ffnw.tile([P, ND, 256], BF16, tag="w1t", bufs=2)
                nc.sync.dma_start(out=w1t, in_=_dram_ap(
                    tn["w1"], fg * 256, [[F, P], [P * F, ND], [1, 256]]))
                for f2 in range(2):
                    ft = fg * 2 + f2
                    a_ps = w1_rotor.tile([P, 512])
                    for idd in range(ND):
                        nc.tensor.matmul(
                            out=a_ps[:], lhsT=w1t[:, idd, f2 * P:(f2 + 1) * P],
                            rhs=xnT[:, idd, tcc * 512:(tcc + 1) * 512],
                            start=(idd == 0), stop=(idd == ND - 1))
                    nc.scalar.activation(
                        out=aT[:, ft, :], in_=a_ps[:],
                        func=AF.Relu, bias=b1_sb[:, ft:ft + 1], scale=1.0)
            for dc in range(2):
                ff_ps = []
                for it4 in range(4):
                    it = tcc * 4 + it4
                    fp = ps_tile(["ff", "ff", "y", "misc"][it4], [P, 512])
                    ff_ps.append(fp)
                    # residual h + b2, folded into the psum accumulation
                    nc.tensor.matmul(
                        out=fp[:], lhsT=ident[:],
                        rhs=h_sb[:, it, dc * 512:(dc + 1) * 512],
                        start=True, stop=False)
                    nc.tensor.matmul(
                        out=fp[:], lhsT=ones1[:, :],
                        rhs=b2_row[:, dc * 512:(dc + 1) * 512],
                        start=False, stop=False)
                for fgrp in range(4):        # 4 groups of 8 f-tiles
                    w2t = ffnw.tile([P, 8, 512], BF16, tag="w2t", bufs=2)
                    nc.sync.dma_start(out=w2t, in_=_dram_ap(
                        tn["w2"], (fgrp * 8 * P) * D + dc * 512,
                        [[D, P], [P * D, 8], [1, 512]]))
                    for fi8 in range(8):
                        ft = fgrp * 8 + fi8
                        last = (fgrp == 3 and fi8 == 7)
                        for it4 in range(4):
                            nc.tensor.matmul(
                                out=ff_ps[it4][:],
                                lhsT=aT[:, ft, it4 * P:(it4 + 1) * P],
                                rhs=w2t[:, fi8, :],
                                start=False, stop=last)
                for it4 in range(4):
                    it = tcc * 4 + it4
                    nc.scalar.copy(
                        out=h_sb[:, it, dc * 512:(dc + 1) * 512],
                        in_=ff_ps[it4][:])
        ffnw.release()

    # ---------------- final LN + vocab projection ----------------
    xnT_bf = big.tile([P, ND, T], BF16, tag="big", name="xnT_bf")
    layer_norm_transpose(xnT_bf)
    voc = tc.alloc_tile_pool(name="voc", bufs=1)
    lg_rotor = TagRotor(["sc", "y", "ff", "misc", "sc", "ff", "y", "ff"])
    for vc in range(NVC):
        wtl = voc.tile([P, ND, VC], BF16, tag="wout", bufs=3)
        nc.sync.dma_start(out=wtl, in_=_dram_ap(
            tn["wout"], vc * VC, [[VSH, P], [P * VSH, ND], [1, VC]]))
        bout_bc = voc.tile([P, VC], F32, tag="bout", bufs=2)
        nc.sync.dma_start(out=bout_bc,
                          in_=_dram_ap(tn["bout"], vc * VC, [[0, P], [1, VC]]))
        lg_sb = voc.tile([P, NT, VC], BF16, tag="lg", bufs=2)
        for it in range(NT):
            lg_ps = lg_rotor.tile([P, VC])
            for idd in range(ND):
                nc.tensor.matmul(
                    out=lg_ps[:], lhsT=xnT_bf[:, idd, it * P:(it + 1) * P],
                    rhs=wtl[:, idd, :],
                    start=(idd == 0), stop=(idd == ND - 1))
            nc.vector.tensor_add(out=lg_sb[:, it, :], in0=lg_ps[:],
                                 in1=bout_bc[:])
        nc.sync.dma_start(
            out=_dram_ap(tn["logits"], vc * VC,
                         [[VSH, P], [P * VSH, NT], [1, VC]]),
            in_=lg_sb[:])
    voc.release()
    for _p in reversed(_static_pools):
        _p.release()


_PROGRAM = None


def _get_program():
    global _PROGRAM
    if _PROGRAM is None:
        _PROGRAM = build_program()
    return _PROGRAM


def make_in_maps(inputs):
    f = lambda k: np.ascontiguousarray(np.asarray(inputs[k], dtype=np.float32))
    x = np.asarray(inputs["x"]).astype(np.int32)          # [B, T]
    import ml_dtypes
    bf = lambda a: np.ascontiguousarray(a.astype(ml_dtypes.bfloat16))

    # fold LN gamma/beta into the consuming weights:
    #   (xhat*g + b) @ W == xhat @ (g[:,None]*W) + b @ W
    g1, b1v = f("ln1_g"), f("ln1_b")
    g2, b2v = f("ln2_g"), f("ln2_b")
    gf, bfv = f("lnf_g"), f("lnf_b")
    wq = f("Wq") * g1[None, :, None]
    wk = f("Wk") * g1[None, :, None]
    wv = f("Wv") * g1[None, :, None]
    bq = f("bq") + np.einsum("d,hdk->hk", b1v, f("Wq"))
    bk = f("bk") + np.einsum("d,hdk->hk", b1v, f("Wk"))
    bv = f("bv") + np.einsum("d,hdk->hk", b1v, f("Wv"))
    w1 = f("W1") * g2[:, None]
    b1 = f("b1") + b2v @ f("W1")
    wout = f("Wout") * gf[:, None]
    bout = f("bout") + bfv @ f("Wout")

    cat = lambda w: np.ascontiguousarray(w.transpose(1, 0, 2).reshape(D, H * KD))
    bpair = lambda b_: np.ascontiguousarray(b_.reshape(NPAIR, P).T.copy())
    wo_aug = np.concatenate([f("Wo"), f("bo").reshape(H, 1, KD)], axis=1)
    wo_aug = np.ascontiguousarray(wo_aug.transpose(1, 0, 2))   # [KD+1, H, KD]
    b1_col = np.ascontiguousarray(b1.reshape(NF, P).T.copy())  # [128, NF]

    shared = {
        "tok_emb": f("tok_emb"), "pos_emb": f("pos_emb"),
        "wq_cat": bf(cat(wq)), "wk_cat": bf(cat(wk)), "wv_cat": bf(cat(wv)),
        "bq_pair": bpair(bq), "bk_pair": bpair(bk),
        "bv_row": np.ascontiguousarray(bv.reshape(1, H * KD)),
        "wo_aug": wo_aug,
        "w1": bf(np.ascontiguousarray(w1)), "b1_col": b1_col,
        "w2": bf(f("W2")), "b2_row": f("b2").reshape(1, D),
    }
    in_maps = []
    for c in range(8):
        b, vh = c // 2, c % 2
        m = dict(shared)
        m["x_idx"] = np.ascontiguousarray(x[b].reshape(T, 1))
        m["wout"] = bf(wout[:, vh * VSH:(vh + 1) * VSH])
        m["bout"] = np.ascontiguousarray(
            bout.reshape(1, V)[:, vh * VSH:(vh + 1) * VSH])
        in_maps.append(m)
    return in_maps


def kernel(**inputs):
    in_maps = make_in_maps(inputs)
    nc = _get_program()
    res = run_bass_kernel_spmd(nc, in_maps, core_ids=list(range(8)))
    out = np.empty((B, T, V), dtype=np.float32)
    for c in range(8):
        b, vh = c // 2, c % 2
        out[b, :, vh * VSH:(vh + 1) * VSH] = \
            np.asarray(res.results[c]["logits"]).astype(np.float32)
    return out

